# revision 5
# baseline (speedup 1.0000x reference)
"""Bass/Trainium2 kernel for FLAOperator(mode='gla') CPU-fallback scan.

Reference recurrence (per b, h, d lane, over t = 0..N-1):
    s_t = s_{t-1} + sigmoid(q_t * k_t + g_t) * v_t ;  y_t = s_t
i.e. y = cumsum over N of u, with u = sigmoid(q*k + g) * v  (pure elementwise).

Shapes: q,k,v,g,y all [B=2, H=16, N=4096, D=128] f32.

Strategy (8 NeuronCores, SPMD, no collectives):
  - Shard the 32 independent (b,h) recurrences: 4 per core.
  - Everything stays in the natural SBUF layout [128 part = n % 128,
    free = (n // 128, d)], so every HBM DMA moves contiguous 512 B rows
    (1 MiB per tensor per 2048-row chunk).
  - u = sigmoid(q*k+g)*v: q*k on GpSimd, +g and *v on DVE, sigmoid on ACT.
  - The cumulative sum is built with the TensorEngine (no transposes):
      * per 128-row block t:  within-block prefix   = Uincl.T @ u_t
        (Uincl[p, m] = 1 for p <= m), batched 4 blocks per matmul (N=512)
      * block offsets: GpSimd partition-reduce gives per-block column sums
        S[t, d]; a small PE matmul with a strict upper-triangular weight
        computes the exclusive prefix over the 32 blocks of each (b,h);
      * a rank-1 matmul (ones[1,128].T @ offsets[1,512]) accumulates the
        offsets into the same PSUM bank, broadcasting along partitions.
  - ACT copies PSUM -> SBUF (DMA cannot read PSUM), then DMA out.
"""

from contextlib import ExitStack

import numpy as np

import concourse.bass as bass
import concourse.tile as tile
from concourse import bacc, mybir
from concourse.bass_utils import run_bass_kernel_spmd

B, H, N, D = 2, 16, 4096, 128
N_CORES = 8
BH = B * H                    # 32 independent recurrences
BH_PER_CORE = BH // N_CORES   # 4
P = 128                       # partitions
CHUNK = 2048                  # n-rows per processing chunk (1 MiB DMAs)
NCHUNKS = N // CHUNK          # 2
GRP = 512                     # columns per matmul group (4 blocks, 1 PSUM bank)
NBLK = N // P                 # 32 blocks per (b, h)
F32 = mybir.dt.float32

_PROGRAM = None       # cached compiled Bass program (module-level)
LAST_RESULTS = None   # BassKernelResults of the last run (for test harness)


def _make_tri(nc, ap, n, strict):
    """ap[p, m] = 1.0 where p < m (strict) or p <= m, else 0.0."""
    nc.gpsimd.memset(ap, 1.0)
    nc.gpsimd.affine_select(
        out=ap,
        in_=ap,
        compare_op=mybir.AluOpType.is_gt if strict else mybir.AluOpType.is_ge,
        fill=0.0,
        base=0,
        pattern=[[1, n]],      # iota = m - p
        channel_multiplier=-1,
    )


def _build_program() -> bass.Bass:
    nc = bacc.Bacc("TRN2", debug=False, num_devices=N_CORES)

    q_d = nc.dram_tensor("q", [BH_PER_CORE, N, D], F32, kind="ExternalInput").ap()
    k_d = nc.dram_tensor("k", [BH_PER_CORE, N, D], F32, kind="ExternalInput").ap()
    v_d = nc.dram_tensor("v", [BH_PER_CORE, N, D], F32, kind="ExternalInput").ap()
    g_d = nc.dram_tensor("g", [BH_PER_CORE, N, D], F32, kind="ExternalInput").ap()
    y_d = nc.dram_tensor("y", [BH_PER_CORE, N, D], F32, kind="ExternalOutput").ap()

    with tile.TileContext(nc) as tc, ExitStack() as ctx:
        const_pool = ctx.enter_context(tc.tile_pool(name="const", bufs=1))
        io_pool = ctx.enter_context(tc.tile_pool(name="io", bufs=2))
        tmp_pool = ctx.enter_context(tc.tile_pool(name="tmp", bufs=2))
        u_pool = ctx.enter_context(tc.tile_pool(name="u", bufs=3))
        s_pool = ctx.enter_context(tc.tile_pool(name="s", bufs=2))
        out_pool = ctx.enter_context(tc.tile_pool(name="out", bufs=2))
        psY_pool = ctx.enter_context(tc.tile_pool(name="psY", bufs=4, space="PSUM"))
        psO_pool = ctx.enter_context(tc.tile_pool(name="psO", bufs=2, space="PSUM"))

        # constants
        u_incl = const_pool.tile([P, P], F32, tag="u_incl")     # p <= m
        _make_tri(nc, u_incl[:], P, strict=False)
        lx32 = const_pool.tile([NBLK, NBLK], F32, tag="lx32")   # p <  m
        _make_tri(nc, lx32[:], NBLK, strict=True)
        ones_row = const_pool.tile([1, P], F32, tag="ones_row")
        nc.vector.memset(ones_row[:], 1.0)

        def dma_in(dst_tile, src_ap):
            # [CHUNK, D] DRAM region -> [128, CHUNK] SBUF tile laid out as
            # partition p = n % 128, free = (n // 128, d); every descriptor
            # moves a contiguous 512 B row.
            nc.sync.dma_start(
                out=dst_tile[:].rearrange("p (t d) -> p t d", d=D),
                in_=src_ap.rearrange("(t p) d -> p t d", p=P),
            )

        for bh in range(BH_PER_CORE):
            us = []      # per-chunk u tiles
            youts = []   # per-chunk staged outputs
            s32 = s_pool.tile([NBLK, P], F32, tag="s32")  # per-block col sums
            for c in range(NCHUNKS):
                rows = slice(c * CHUNK, (c + 1) * CHUNK)
                qt = io_pool.tile([P, CHUNK], F32, tag="q")
                kt = io_pool.tile([P, CHUNK], F32, tag="k")
                vt = io_pool.tile([P, CHUNK], F32, tag="v")
                gt = io_pool.tile([P, CHUNK], F32, tag="g")
                dma_in(qt, q_d[bh, rows, :])
                dma_in(kt, k_d[bh, rows, :])
                dma_in(vt, v_d[bh, rows, :])
                dma_in(gt, g_d[bh, rows, :])

                # u = sigmoid(q*k + g) * v
                qk = tmp_pool.tile([P, CHUNK], F32, tag="qk")
                nc.gpsimd.tensor_mul(qk[:], qt[:], kt[:])
                a = tmp_pool.tile([P, CHUNK], F32, tag="a")
                nc.vector.tensor_add(a[:], qk[:], gt[:])
                nc.scalar.activation(a[:], a[:], mybir.ActivationFunctionType.Sigmoid)
                ut = u_pool.tile([P, CHUNK], F32, tag="u")
                nc.vector.tensor_mul(ut[:], a[:], vt[:])
                us.append(ut)

                # per-block column sums for this chunk: S[t, d] = sum_p u[p,t,d]
                s_row = tmp_pool.tile([1, CHUNK], F32, tag="s_row")
                nc.gpsimd.tensor_reduce(
                    s_row[:], ut[:], axis=mybir.AxisListType.C, op=mybir.AluOpType.add
                )
                # scatter [1, (t d)] -> rows 16c..16c+15 of s32
                nc.sync.dma_start(
                    out=s32[c * (CHUNK // P) : (c + 1) * (CHUNK // P), :],
                    in_=s_row[:].rearrange("p (t d) -> p t d", d=D),
                )

            # exclusive prefix over the 32 block sums: offs[m, d] = sum_{t<m} S[t, d]
            offs_ps = psO_pool.tile([NBLK, P], F32, tag="offs_ps")
            nc.tensor.matmul(offs_ps[:], lx32[:], s32[:])
            offs = s_pool.tile([NBLK, P], F32, tag="offs")
            nc.scalar.copy(offs[:], offs_ps[:])
            # flatten [32, 128] -> one row [1, (t d)] so rank-1 matmuls can read it
            offs_flat = s_pool.tile([1, NBLK * P], F32, tag="offs_flat")
            nc.sync.dma_start(
                out=offs_flat[:].rearrange("p (t d) -> p t d", d=D),
                in_=offs[:],
            )

            for c in range(NCHUNKS):
                yout = out_pool.tile([P, CHUNK], F32, tag="yout")
                youts.append(yout)
                for s in range(CHUNK // GRP):
                    g = c * (CHUNK // GRP) + s   # global group index (0..7)
                    lo = s * GRP
                    ps = psY_pool.tile([P, GRP], F32, tag="psY")
                    # within-block inclusive prefix for 4 blocks at once
                    nc.tensor.matmul(
                        ps[:], u_incl[:], us[c][:, lo : lo + GRP],
                        start=True, stop=False,
                    )
                    # + exclusive block offsets, broadcast down partitions
                    nc.tensor.matmul(
                        ps[:], ones_row[:],
                        offs_flat[:, g * GRP : (g + 1) * GRP],
                        start=False, stop=True,
                    )
                    nc.scalar.copy(yout[:, lo : lo + GRP], ps[:])

                nc.sync.dma_start(
                    out=y_d[bh, c * CHUNK : (c + 1) * CHUNK, :].rearrange(
                        "(t p) d -> p t d", p=P
                    ),
                    in_=yout[:].rearrange("p (t d) -> p t d", d=D),
                )

    nc.compile()  # bacc backend: wait legalization, reg alloc, nop fusion
    return nc


def kernel(q: np.ndarray, k: np.ndarray, v: np.ndarray, g: np.ndarray) -> np.ndarray:
    global _PROGRAM, LAST_RESULTS
    if _PROGRAM is None:
        _PROGRAM = _build_program()

    def shard(x):
        x = np.ascontiguousarray(np.asarray(x, dtype=np.float32)).reshape(BH, N, D)
        return [np.ascontiguousarray(x[i * BH_PER_CORE : (i + 1) * BH_PER_CORE])
                for i in range(N_CORES)]

    qs, ks, vs, gs = shard(q), shard(k), shard(v), shard(g)
    in_maps = [
        {"q": qs[i], "k": ks[i], "v": vs[i], "g": gs[i]} for i in range(N_CORES)
    ]
    LAST_RESULTS = run_bass_kernel_spmd(_PROGRAM, in_maps, core_ids=list(range(N_CORES)))
    y = np.concatenate([r["y"] for r in LAST_RESULTS.results], axis=0)
    return y.reshape(B, H, N, D)


# revision 7
# speedup vs baseline: 10.3242x; 10.3242x over previous
"""Bass/Trainium2 kernel for FLAOperator(mode='gla') CPU-fallback scan.

Reference recurrence (per b, h, d lane, over t = 0..N-1):
    s_t = s_{t-1} + sigmoid(q_t * k_t + g_t) * v_t ;  y_t = s_t
i.e. y = cumsum over N of u, with u = sigmoid(q*k + g) * v  (pure elementwise).

Shapes: q,k,v,g,y all [B=2, H=16, N=4096, D=128] f32.

Strategy (8 NeuronCores, SPMD, no collectives):
  - Shard the 32 independent (b,h) recurrences: 4 per core.
  - Everything stays in the natural SBUF layout [128 part = n % 128,
    free = (n // 128, d)], so every HBM DMA moves contiguous 512 B rows
    (1 MiB per tensor per 2048-row chunk).
  - u = sigmoid(q*k+g)*v on DVE (mult, add, mult) + ACT (sigmoid).
  - The cumulative sum is built on the TensorEngine (no transposes):
      * MM1 per group of 4 blocks (N=512): psum = Uincl.T @ u
        (Uincl[p, m] = 1 for p <= m) -> within-block inclusive prefixes;
        row 127 of the result is the per-block column sum for free.
      * ACT copies row 127 into an SBUF row; a small PE matmul with a
        strict upper-triangular [16, 17] weight turns the 16 block sums
        of a chunk into exclusive block offsets (row 16 = chunk total);
        the carry from the previous chunk is added with a rank-1 matmul.
      * rank-1 matmuls (ones[1,128].T @ offsets[1,512]) accumulate the
        offsets into each group's PSUM, broadcasting along partitions.
  - ACT copies PSUM -> SBUF (DMA cannot read PSUM), then DMA out.
"""

from contextlib import ExitStack

import numpy as np

import concourse.bass as bass
import concourse.tile as tile
from concourse import bacc, mybir
from concourse.bass_utils import run_bass_kernel_spmd

B, H, N, D = 2, 16, 4096, 128
N_CORES = 8
BH = B * H                    # 32 independent recurrences
BH_PER_CORE = BH // N_CORES   # 4
P = 128                       # partitions
CHUNK = 2048                  # n-rows per processing chunk (1 MiB DMAs)
NCHUNKS = N // CHUNK          # 2
GRP = 512                     # columns per matmul group (4 blocks, 1 PSUM bank)
TPC = CHUNK // P              # blocks per chunk (16)
F32 = mybir.dt.float32

_PROGRAM = None       # cached compiled Bass program (module-level)
LAST_RESULTS = None   # BassKernelResults of the last run (for test harness)


def _make_tri(nc, ap, ncols, strict):
    """ap[p, m] = 1.0 where p < m (strict) or p <= m, else 0.0."""
    nc.gpsimd.memset(ap, 1.0)
    nc.gpsimd.affine_select(
        out=ap,
        in_=ap,
        compare_op=mybir.AluOpType.is_gt if strict else mybir.AluOpType.is_ge,
        fill=0.0,
        base=0,
        pattern=[[1, ncols]],      # iota = m - p
        channel_multiplier=-1,
    )


def _build_program() -> bass.Bass:
    nc = bacc.Bacc("TRN2", debug=False, num_devices=N_CORES)

    q_d = nc.dram_tensor("q", [BH_PER_CORE, N, D], F32, kind="ExternalInput").ap()
    k_d = nc.dram_tensor("k", [BH_PER_CORE, N, D], F32, kind="ExternalInput").ap()
    v_d = nc.dram_tensor("v", [BH_PER_CORE, N, D], F32, kind="ExternalInput").ap()
    g_d = nc.dram_tensor("g", [BH_PER_CORE, N, D], F32, kind="ExternalInput").ap()
    y_d = nc.dram_tensor("y", [BH_PER_CORE, N, D], F32, kind="ExternalOutput").ap()

    with tile.TileContext(nc) as tc, ExitStack() as ctx:
        const_pool = ctx.enter_context(tc.tile_pool(name="const", bufs=1))
        io_pool = ctx.enter_context(tc.tile_pool(name="io", bufs=2))
        tmp_pool = ctx.enter_context(tc.tile_pool(name="tmp", bufs=2))
        u_pool = ctx.enter_context(tc.tile_pool(name="u", bufs=2))
        s_pool = ctx.enter_context(tc.tile_pool(name="s", bufs=2))
        out_pool = ctx.enter_context(tc.tile_pool(name="out", bufs=2))
        psY_pool = ctx.enter_context(tc.tile_pool(name="psY", bufs=5, space="PSUM"))
        psO_pool = ctx.enter_context(tc.tile_pool(name="psO", bufs=2, space="PSUM"))

        # constants
        u_incl = const_pool.tile([P, P], F32, tag="u_incl")       # p <= m
        _make_tri(nc, u_incl[:], P, strict=False)
        lx17 = const_pool.tile([TPC, TPC + 1], F32, tag="lx17")   # p <  m
        _make_tri(nc, lx17[:], TPC + 1, strict=True)
        ones_row = const_pool.tile([1, P], F32, tag="ones_row")
        nc.vector.memset(ones_row[:], 1.0)
        ones17 = const_pool.tile([1, TPC + 1], F32, tag="ones17")
        nc.vector.memset(ones17[:], 1.0)

        def dma_in(dst_tile, src_ap):
            # [CHUNK, D] DRAM region -> [128, CHUNK] SBUF tile laid out as
            # partition p = n % 128, free = (n // 128, d); every descriptor
            # moves a contiguous 512 B row.
            nc.sync.dma_start(
                out=dst_tile[:].rearrange("p (t d) -> p t d", d=D),
                in_=src_ap.rearrange("(t p) d -> p t d", p=P),
            )

        for bh in range(BH_PER_CORE):
            g_tile = None  # [1, 128] carry into the current chunk (None = zero)
            for c in range(NCHUNKS):
                rows = slice(c * CHUNK, (c + 1) * CHUNK)
                qt = io_pool.tile([P, CHUNK], F32, tag="q")
                kt = io_pool.tile([P, CHUNK], F32, tag="k")
                vt = io_pool.tile([P, CHUNK], F32, tag="v")
                gt = io_pool.tile([P, CHUNK], F32, tag="g")
                dma_in(qt, q_d[bh, rows, :])
                dma_in(kt, k_d[bh, rows, :])
                dma_in(vt, v_d[bh, rows, :])
                dma_in(gt, g_d[bh, rows, :])

                # u = sigmoid(q*k + g) * v
                a = tmp_pool.tile([P, CHUNK], F32, tag="a")
                nc.vector.tensor_mul(a[:], qt[:], kt[:])
                nc.vector.tensor_add(a[:], a[:], gt[:])
                nc.scalar.activation(a[:], a[:], mybir.ActivationFunctionType.Sigmoid)
                ut = u_pool.tile([P, CHUNK], F32, tag="u")
                nc.vector.tensor_mul(ut[:], a[:], vt[:])

                # MM1 per group: within-block prefixes; row 127 = block sums
                s_row = tmp_pool.tile([P, CHUNK], F32, tag="s_row")  # row 127 used
                pss = []
                for s in range(CHUNK // GRP):
                    lo = s * GRP
                    ps = psY_pool.tile([P, GRP], F32, tag="psY")
                    pss.append(ps)
                    nc.tensor.matmul(
                        ps[:], u_incl[:], ut[:, lo : lo + GRP],
                        start=True, stop=False, skip_group_check=True,
                    )
                    # PSUM partition base must be quadrant-aligned: copy the
                    # last 32 rows; only row 127 (the block sums) is used.
                    nc.scalar.copy(
                        s_row[96:P, lo : lo + GRP], ps[96:P, :]
                    )

                # 16 block sums -> exclusive offsets (+ carry), as one SBUF row
                s16 = s_pool.tile([TPC, P], F32, tag="s16")
                nc.sync.dma_start(
                    out=s16[:],
                    in_=s_row[P - 1 : P, :].rearrange("p (t d) -> p t d", d=D),
                )
                offs_ps = psO_pool.tile([TPC + 1, P], F32, tag="offs_ps")
                last = c == NCHUNKS - 1
                nc.tensor.matmul(
                    offs_ps[:], lx17[:], s16[:],
                    start=True, stop=(g_tile is None), skip_group_check=True,
                )
                if g_tile is not None:
                    nc.tensor.matmul(
                        offs_ps[:], ones17[:], g_tile[:],
                        start=False, stop=True, skip_group_check=True,
                    )
                offs = s_pool.tile([TPC + 1, P], F32, tag="offs")
                nc.scalar.copy(offs[:], offs_ps[:])
                offs_flat = s_pool.tile([1, CHUNK], F32, tag="offs_flat")
                nc.sync.dma_start(
                    out=offs_flat[:].rearrange("p (t d) -> p t d", d=D),
                    in_=offs[0:TPC, :],
                )
                if not last:
                    ng = s_pool.tile([1, P], F32, tag="g_carry")
                    nc.sync.dma_start(out=ng[:], in_=offs[TPC : TPC + 1, :])
                    g_tile = ng
                else:
                    g_tile = None

                # add offsets (broadcast down partitions) and stage the output
                yout = out_pool.tile([P, CHUNK], F32, tag="yout")
                for s in range(CHUNK // GRP):
                    lo = s * GRP
                    nc.tensor.matmul(
                        pss[s][:], ones_row[:], offs_flat[:, lo : lo + GRP],
                        start=False, stop=True, skip_group_check=True,
                    )
                    nc.scalar.copy(yout[:, lo : lo + GRP], pss[s][:])

                nc.sync.dma_start(
                    out=y_d[bh, rows, :].rearrange("(t p) d -> p t d", p=P),
                    in_=yout[:].rearrange("p (t d) -> p t d", d=D),
                )

    nc.compile()  # bacc backend: wait legalization, reg alloc, nop fusion
    return nc


def kernel(q: np.ndarray, k: np.ndarray, v: np.ndarray, g: np.ndarray) -> np.ndarray:
    global _PROGRAM, LAST_RESULTS
    if _PROGRAM is None:
        _PROGRAM = _build_program()

    def shard(x):
        x = np.ascontiguousarray(np.asarray(x, dtype=np.float32)).reshape(BH, N, D)
        return [np.ascontiguousarray(x[i * BH_PER_CORE : (i + 1) * BH_PER_CORE])
                for i in range(N_CORES)]

    qs, ks, vs, gs = shard(q), shard(k), shard(v), shard(g)
    in_maps = [
        {"q": qs[i], "k": ks[i], "v": vs[i], "g": gs[i]} for i in range(N_CORES)
    ]
    LAST_RESULTS = run_bass_kernel_spmd(_PROGRAM, in_maps, core_ids=list(range(N_CORES)))
    y = np.concatenate([r["y"] for r in LAST_RESULTS.results], axis=0)
    return y.reshape(B, H, N, D)
